# revision 4
# baseline (speedup 1.0000x reference)
"""CrossOnlyAttention Trainium2 kernel (v2).

Data-parallel over batch: 64 batches -> 8 cores x 8 batches. fp16 matmuls.

Per-core dataflow (per batch):
  x^T [C,T] in SBUF -> QKV projections:
     Q^T,K^T head-major [2 heads x 64, T] per head-pair (bias + 1/8 scale fused
     into the PSUM->SBUF eviction on DVE), V token-major [T, 64] per head with
     a ones column appended (V_aug) so the PV matmul also produces the softmax
     denominator Z as row 64.
  Scores S^T[k,q] = K^T.T @ Q^T (contraction=64 on PE rows 0:64 / 64:128 per
     head parity). The additive cross-mask is rank-2 in (k, img(q)):
       bias = a1(q)*(kzero+img2)(k) + a2(q)*(1-img2)(k)
     with a1(q)=[1<=q<235], a2(q)=[q>=235] (both 0 at q=0, where the true bias
     is constant over k and softmax-cancels). It is added by a second K=2
     matmul (lhsT=kaug, rhs=qaug) accumulating into the same PSUM bank on the
     opposite 2 PE rows (disjoint row group -> runs concurrently, ~free).
  exp: ONE plain activation per (h,kt) tile (no bias operand, no slicing).
  PV: Y^T[66,q] = V_aug.T @ E accumulated over 4 k-tiles; row 64 = Z.
  Normalize: R=1/Z via reciprocal_approx_fast (fp32) + f16 cast, broadcast R
     across 64 partitions with a K=1 matmul, DVE multiply (reading both pv and
     bc straight from PSUM) -> Yhat^T [C,T]; odd heads shifted to partitions
     64:128 via SBUF->SBUF DMA. V-bias is folded into b_proj on the host.
  Proj: out[t, c] accumulated over 8 cin tiles; b_proj added during the DVE
     eviction from a broadcast bias tile (built once by two K=1 matmuls).

Emission pipeline: PV of head h is deferred until after the NEXT head-pair's
QK matmuls are emitted, so the Act-engine exp chain of heads h, h+1 hides
under ~3.2us of independent PE work and the PE never idles long enough for
the HAM clock gate to re-throttle.

fp16 ISA restrictions (walrus s3d3_mm): every matmul operand free-dim count
and the PSUM dst free count must be EVEN, dst start_partition must be 0.
Token free dims padded 469->470 (TE), ragged stationary slices 85->86 (STT).
"""

import os
import sys

import numpy as np

for _p in (
    "/opt/trn_rl_repo",
    "/root/.axon_site",
    "/root/.axon_site/_ro/trn_rl_repo",
    "/root/.axon_site/_ro/pypackages",
):
    if os.path.isdir(_p) and _p not in sys.path:
        sys.path.append(_p)

import concourse.bass as bass  # noqa: E402,F401
import concourse.tile as tile  # noqa: E402
from concourse import bacc, mybir  # noqa: E402
from concourse.bass_utils import run_bass_kernel_spmd  # noqa: E402

B, T, C = 64, 469, 1024
H, HD = 16, 64
T1 = 234
NCORES = 8
BL = B // NCORES
F16 = mybir.dt.float16
F32 = mybir.dt.float32
TT = [128, 128, 128, 85]    # real token-tile sizes
STT = [128, 128, 128, 86]   # even-padded stationary slice sizes
TOFF = [0, 128, 256, 384]
TE = 470                    # even-padded T for matmul free dims
NKT = 8  # cin contraction tiles (1024/128)
EXP = mybir.ActivationFunctionType.Exp
MUL = mybir.AluOpType.mult
ADD = mybir.AluOpType.add

_cache = {}


def _build():
    nc = bacc.Bacc(trn_type="TRN2", name="xattn")
    x_h = nc.dram_tensor("x", [BL, C, T], F16, kind="ExternalInput")
    wqk_h = nc.dram_tensor("wqk", [128, NKT, 2 * C], F16, kind="ExternalInput")
    wv_h = nc.dram_tensor("wv", [4, 128, NKT, 256], F16, kind="ExternalInput")
    wp_h = nc.dram_tensor("wp", [4, 128, NKT, 256], F16, kind="ExternalInput")
    bqs_h = nc.dram_tensor("bqs", [C], F32, kind="ExternalInput")
    bk_h = nc.dram_tensor("bk", [C], F32, kind="ExternalInput")
    bpe_h = nc.dram_tensor("bpe", [C], F16, kind="ExternalInput")
    kaug_h = nc.dram_tensor("kaug", [128, TE], F16, kind="ExternalInput")
    qaug_h = nc.dram_tensor("qaug", [128, TE], F16, kind="ExternalInput")
    out_h = nc.dram_tensor("out", [BL, T, C], F32, kind="ExternalOutput")

    with tile.TileContext(nc) as tc:
        with (
            tc.tile_pool(name="singles", bufs=1) as singles,
            tc.tile_pool(name="xy", bufs=2) as xy_pool,
            tc.tile_pool(name="qk", bufs=4) as qk_pool,
            tc.tile_pool(name="ep", bufs=9) as e_pool,
            tc.tile_pool(name="vw", bufs=2) as vw_pool,
            tc.tile_pool(name="pw", bufs=2) as pw_pool,
            tc.tile_pool(name="rp", bufs=2) as r_pool,
            tc.tile_pool(name="rf", bufs=2) as rf_pool,
            tc.tile_pool(name="tp", bufs=2) as t_pool,
            tc.tile_pool(name="yp", bufs=4) as y_pool,
            tc.tile_pool(name="acc", bufs=2, space="PSUM") as acc_pool,
            tc.tile_pool(name="sp", bufs=4, space="PSUM") as s_pool,
            tc.tile_pool(name="pvp", bufs=2, space="PSUM") as pv_pool,
        ):
            # ---- resident constants ----
            wqk = singles.tile([128, NKT, 2 * C], F16)
            nc.sync.dma_start(wqk[:], wqk_h[:])
            ones32 = singles.tile([128, 128], F32)
            nc.vector.memset(ones32[:], 1.0)
            ones = singles.tile([128, 128], F16)
            nc.scalar.copy(ones[:], ones32[:])
            zeros32 = singles.tile([128, NKT], F32)
            nc.vector.memset(zeros32[:], 0.0)
            onz = singles.tile([128, H, 2], F32)
            nc.vector.memset(onz[:, :, 0:1], 1.0)
            nc.vector.memset(onz[:, :, 1:2], 0.0)
            vsb = singles.tile([128, 4, H, 66], F16)
            for _tt in range(4):
                nc.scalar.copy(vsb[:, _tt, :, 64:66], onz[:])
            bqs = singles.tile([128, NKT], F32)
            nc.sync.dma_start(bqs[:], bqs_h.ap().rearrange("(cb p) -> p cb", p=128))
            bk = singles.tile([128, NKT], F32)
            nc.sync.dma_start(bk[:], bk_h.ap().rearrange("(cb p) -> p cb", p=128))
            bpe = singles.tile([1, C], F16)
            nc.sync.dma_start(bpe[:], bpe_h.ap().unsqueeze(0))
            kaug = singles.tile([128, TE], F16)
            nc.sync.dma_start(kaug[:], kaug_h[:])
            qaug = singles.tile([128, TE], F16)
            nc.sync.dma_start(qaug[:], qaug_h[:])
            # broadcast b_proj_eff across all 128 partitions (once)
            bpb = singles.tile([128, C], F32)
            for half in range(2):
                bacc_ps = acc_pool.tile([128, 512], F32, tag="acc")
                nc.tensor.matmul(
                    bacc_ps[:, 0:512],
                    lhsT=ones[0:1, 0:128],
                    rhs=bpe[0:1, 512 * half : 512 * (half + 1)],
                    start=True,
                    stop=True,
                )
                nc.vector.tensor_copy(
                    bpb[:, 512 * half : 512 * (half + 1)], bacc_ps[:, 0:512]
                )

            for b in range(BL):
                xT = xy_pool.tile([128, NKT, TE], F16, tag="xy")
                for cs in range(NKT):
                    nc.sync.dma_start(
                        xT[:, cs, 0:T], x_h[b][128 * cs : 128 * (cs + 1), :]
                    )
                nc.scalar.copy(xT[:, :, T:TE], zeros32[:].unsqueeze(2))
                yh = xy_pool.tile([128, NKT, TE], F16, tag="xy")
                nc.scalar.copy(yh[:, :, T:TE], zeros32[:].unsqueeze(2))

                pending = []  # heads with S+exp emitted, PV deferred
                state = {"pend": None}

                def norm_tail(pv, r, h, yh=yh):
                    hp, sub = h // 2, h % 2
                    bc = s_pool.tile([128, TE], F32, tag="s")
                    nc.tensor.matmul(
                        bc[0:64, 0:TE],
                        lhsT=ones[64:65, 0:64],
                        rhs=r[64:65, 0:TE],
                        start=True,
                        stop=True,
                    )
                    bcs = t_pool.tile([64, T], F32, tag="bcs")
                    nc.scalar.copy(bcs[:, 0:T], bc[0:64, 0:T])
                    if sub == 0:
                        nc.vector.tensor_mul(
                            yh[0:64, hp, 0:T], pv[0:64, 0:T], bcs[:, 0:T]
                        )
                    else:
                        tmp = t_pool.tile([64, T], F16, tag="tmp")
                        nc.vector.tensor_mul(
                            tmp[:, 0:T], pv[0:64, 0:T], bcs[:, 0:T]
                        )
                        nc.sync.dma_start(yh[64:128, hp, 0:T], tmp[:, 0:T])

                def flush_pv():
                    h, etiles = pending.pop(0)
                    pv = pv_pool.tile([66, TE], F32, tag="pv")
                    for kt in range(4):
                        n = TT[kt]
                        nc.tensor.matmul(
                            pv[:, 0:TE],
                            lhsT=vsb[0:n, kt, h, 0:66],
                            rhs=etiles[kt][0:n, 0:TE],
                            start=(kt == 0),
                            stop=(kt == 3),
                        )
                    r = r_pool.tile([65, TE], F16, tag="r")
                    with nc.allow_low_precision(
                        reason="softmax denominators fit fp16"
                    ):
                        nc.vector.reciprocal(r[64:65, 0:TE], pv[64:65, 0:TE])
                    if state["pend"] is not None:
                        norm_tail(*state["pend"])
                    state["pend"] = (pv, r, h)

                def emit_s(h, qsb, ksb):
                    r0 = 64 * (h % 2)
                    a0 = 64 - r0
                    etiles = []
                    for kt in range(4):
                        n, sn = TT[kt], STT[kt]
                        ko = TOFF[kt]
                        s = s_pool.tile([128, TE], F32, tag="s")
                        nc.tensor.matmul(
                            s[0:sn, 0:TE],
                            lhsT=ksb[r0 : r0 + 64, ko : ko + sn],
                            rhs=qsb[r0 : r0 + 64, 0:TE],
                            start=True,
                            stop=False,
                        )
                        nc.tensor.matmul(
                            s[0:sn, 0:TE],
                            lhsT=kaug[a0 : a0 + 2, ko : ko + sn],
                            rhs=qaug[a0 : a0 + 2, 0:TE],
                            start=False,
                            stop=True,
                        )
                        e = e_pool.tile([128, TE], F16, tag="e")
                        nc.scalar.activation(e[0:n, 0:TE], s[0:n, 0:TE], EXP)
                        etiles.append(e)
                    pending.append((h, etiles))

                for c in range(4):
                    # V projection chunk: heads 4c..4c+3 (vcols 256c..256c+256)
                    vw = vw_pool.tile([128, NKT, 256], F16, tag="vw")
                    nc.sync.dma_start(vw[:], wv_h[c])
                    for tt in range(4):
                        n, sn = TT[tt], STT[tt]
                        acc = acc_pool.tile([128, TE], F32, tag="acc")
                        for kt in range(NKT):
                            nc.tensor.matmul(
                                acc[0:sn, 0:256],
                                lhsT=xT[:, kt, TOFF[tt] : TOFF[tt] + sn],
                                rhs=vw[:, kt, :],
                                start=(kt == 0),
                                stop=(kt == NKT - 1),
                            )
                        nc.vector.tensor_copy(
                            vsb[0:n, tt, 4 * c : 4 * c + 4, 0:64],
                            acc[0:n, 0:256].rearrange("p (h d) -> p h d", h=4),
                        )

                    for hp in (2 * c, 2 * c + 1):
                        # Q^T / K^T for head pair hp (heads 2hp, 2hp+1)
                        qacc = acc_pool.tile([128, TE], F32, tag="acc")
                        for kt in range(NKT):
                            nc.tensor.matmul(
                                qacc[:, 0:TE],
                                lhsT=wqk[:, kt, 128 * hp : 128 * hp + 128],
                                rhs=xT[:, kt, 0:TE],
                                start=(kt == 0),
                                stop=(kt == NKT - 1),
                            )
                        qsb = qk_pool.tile([128, TE], F16, tag="qk")
                        nc.vector.tensor_scalar(
                            qsb[:, 0:TE], qacc[:, 0:TE],
                            0.125, bqs[:, hp : hp + 1], MUL, ADD,
                        )
                        kacc = acc_pool.tile([128, TE], F32, tag="acc")
                        for kt in range(NKT):
                            nc.tensor.matmul(
                                kacc[:, 0:TE],
                                lhsT=wqk[:, kt, C + 128 * hp : C + 128 * hp + 128],
                                rhs=xT[:, kt, 0:TE],
                                start=(kt == 0),
                                stop=(kt == NKT - 1),
                            )
                        ksb = qk_pool.tile([128, TE], F16, tag="qk")
                        nc.vector.tensor_scalar_add(
                            ksb[:, 0:TE], kacc[:, 0:TE], bk[:, hp : hp + 1]
                        )

                        for sub in range(2):
                            h = 2 * hp + sub
                            if len(pending) >= 2:
                                flush_pv()
                            emit_s(h, qsb, ksb)

                while pending:
                    flush_pv()
                if state["pend"] is not None:
                    norm_tail(*state["pend"])
                    state["pend"] = None

                # ---- output projection ----
                ytiles = []
                for _tt in range(4):
                    ytile = y_pool.tile([128, C], F32, tag="y", name=f"y{b}_{_tt}")
                    ytiles.append(ytile)
                for ch in range(4):
                    pw = pw_pool.tile([128, NKT, 256], F16, tag="pw")
                    nc.sync.dma_start(pw[:], wp_h[ch])
                    for tt in range(4):
                        n, sn = TT[tt], STT[tt]
                        acc = acc_pool.tile([128, TE], F32, tag="acc")
                        for kt in range(NKT):
                            nc.tensor.matmul(
                                acc[0:sn, 0:256],
                                lhsT=yh[:, kt, TOFF[tt] : TOFF[tt] + sn],
                                rhs=pw[:, kt, :],
                                start=(kt == 0),
                                stop=(kt == NKT - 1),
                            )
                        nc.vector.tensor_add(
                            ytiles[tt][0:n, 256 * ch : 256 * (ch + 1)],
                            acc[0:n, 0:256],
                            bpb[0:n, 256 * ch : 256 * (ch + 1)],
                        )
                for tt in range(4):
                    n = TT[tt]
                    nc.sync.dma_start(
                        out_h[b, TOFF[tt] : TOFF[tt] + n, :], ytiles[tt][0:n, :]
                    )

    nc.compile()
    return nc


def _pack_w(w):
    # [C, n] -> [128, NKT, n] with w_packed[p, kt, j] = w[kt*128 + p, j]
    n = w.shape[1]
    return np.ascontiguousarray(
        w.reshape(NKT, 128, n).transpose(1, 0, 2), dtype=np.float16
    )


def _prep_inputs(x, W_attn, b_attn, W_proj, b_proj):
    wqk = _pack_w(np.asarray(W_attn[:, : 2 * C]))
    wv = np.stack(
        [
            _pack_w(np.asarray(W_attn[:, 2 * C + 256 * c : 2 * C + 256 * (c + 1)]))
            for c in range(4)
        ]
    )
    wpk = np.stack(
        [
            _pack_w(np.asarray(W_proj[:, 256 * c : 256 * (c + 1)]))
            for c in range(4)
        ]
    )
    bqs = (b_attn[:C].astype(np.float64) * 0.125).astype(np.float32)
    bk = np.ascontiguousarray(b_attn[C : 2 * C], dtype=np.float32)
    bv = b_attn[2 * C :].astype(np.float64)
    bpe = (b_proj.astype(np.float64) + bv @ W_proj.astype(np.float64)).astype(
        np.float16
    )
    # rank-2 additive mask: bias(k,q) = a1(q)*row0(k) + a2(q)*row1(k)
    k = np.arange(TE)
    img2 = ((k >= T1 + 1) & (k < T)).astype(np.float32)
    kzero = (k == 0).astype(np.float32)
    row0 = kzero + img2            # q in img1: mask 1 at k=0 and k in img2
    row1 = (k < T1 + 1).astype(np.float32)  # q in img2: 1 at k=0 and k in img1
    kaug = np.zeros((128, TE), dtype=np.float16)
    qaug = np.zeros((128, TE), dtype=np.float16)
    q = np.arange(TE)
    a1 = ((q >= 1) & (q < T1 + 1)).astype(np.float32)
    a2 = ((q >= T1 + 1) & (q < T)).astype(np.float32)
    for base in (0, 64):
        kaug[base + 0, :] = row0
        kaug[base + 1, :] = row1
        qaug[base + 0, :] = a1
        qaug[base + 1, :] = a2
    common = {
        "wqk": wqk, "wv": wv, "wp": wpk, "bqs": bqs, "bk": bk,
        "bpe": bpe, "kaug": kaug, "qaug": qaug,
    }
    # x -> [B, C, T] fp16 (pre-transposed so the device DMA is near-linear)
    xs = np.ascontiguousarray(
        np.asarray(x).astype(np.float16).transpose(0, 2, 1)
    )
    in_maps = []
    for cidx in range(NCORES):
        m = dict(common)
        m["x"] = np.ascontiguousarray(xs[cidx * BL : (cidx + 1) * BL])
        in_maps.append(m)
    return in_maps


def _run(x, W_attn, b_attn, W_proj, b_proj, trace=False):
    if "nc" not in _cache:
        _cache["nc"] = _build()
    nc = _cache["nc"]
    in_maps = _prep_inputs(x, W_attn, b_attn, W_proj, b_proj)
    res = run_bass_kernel_spmd(
        nc, in_maps, core_ids=list(range(NCORES)), trace=trace
    )
    out = np.concatenate([r["out"] for r in res.results], axis=0)
    return out.astype(np.float32), res


def kernel(x, W_attn, b_attn, W_proj, b_proj):
    out, _ = _run(x, W_attn, b_attn, W_proj, b_proj, trace=False)
    return out


# revision 6
# speedup vs baseline: 1.2387x; 1.2387x over previous
"""CrossOnlyAttention Trainium2 kernel (v3).

Data-parallel over batch: 64 batches -> 8 cores x 8 batches. fp16 matmuls.

Per-core dataflow (per batch):
  x^T [C,T] in SBUF -> QKV projections:
     Q^T,K^T head-major [2 heads x 64, T] per head-pair (bias + 1/8 scale fused
     into the PSUM->SBUF eviction on DVE), V token-major [T, 64] per head with
     a ones column appended (V_aug) so the PV matmul also produces the softmax
     denominator Z as row 64.
  Scores S^T[k,q] = K^T.T @ Q^T (contraction=64 on PE rows 0:64 / 64:128 per
     head parity). The additive cross-mask is rank-2 in (k, img(q)):
       bias = a1(q)*(kzero+img2)(k) + a2(q)*(1-img2)(k)
     with a1(q)=[1<=q<235], a2(q)=[q>=235] (both 0 at q=0, where the true bias
     is constant over k and softmax-cancels). It is added by a second K=2
     matmul (lhsT=kaug, rhs=qaug) accumulating into the same PSUM bank on the
     opposite 2 PE rows (disjoint row group -> overlaps the main matmul).
  exp: score k-tiles are paired into 2-bank PSUM units so ONE plain activation
     covers two tiles (amortizes the ~293ns ACT fixed overhead; no bias
     operand, no free-dim slicing).
  PV: Y^T[66,q] = V_aug.T @ E accumulated over 4 k-tiles; row 64 = Z.
  Normalize: R = exp(-ln Z) on the Act engine (both fns in one LUT table set;
     reads Z straight from PSUM, writes f16), broadcast R across 64 partitions
     with a K=1 matmul into an acc-pool bank, copy to SBUF, DVE multiply ->
     Yhat^T [C,T]; odd heads shifted to partitions 64:128 via SBUF->SBUF DMA.
     V-bias is folded into b_proj on the host.
  Proj: out[t, c] accumulated over 8 cin tiles at N=512; b_proj added during
     the DVE eviction from a broadcast bias tile (built once by K=1 matmuls).

Emission pipeline per head-pair: QKq(hp) | PV(h-2) | S(h0) | QKk(hp) |
PV(h-1) | S(h1), with norm tails deferred one head. Dependency-gated matmuls
(PV, S) are thereby separated from their producers by ~1.6us of independent
QK work, so the PE never idles long enough for the HAM clock gate to
re-throttle.

fp16 ISA restrictions (walrus s3d3_mm): every matmul operand free-dim count
and the PSUM dst free count must be EVEN, dst start_partition must be 0.
Token free dims padded 469->470 (TE), ragged stationary slices 85->86 (STT).
"""

import os
import sys

import numpy as np

for _p in (
    "/opt/trn_rl_repo",
    "/root/.axon_site",
    "/root/.axon_site/_ro/trn_rl_repo",
    "/root/.axon_site/_ro/pypackages",
):
    if os.path.isdir(_p) and _p not in sys.path:
        sys.path.append(_p)

import concourse.bass as bass  # noqa: E402,F401
import concourse.tile as tile  # noqa: E402
from concourse import bacc, mybir  # noqa: E402
from concourse.bass_utils import run_bass_kernel_spmd  # noqa: E402

B, T, C = 64, 469, 1024
H, HD = 16, 64
T1 = 234
NCORES = 8
BL = B // NCORES
F16 = mybir.dt.float16
F32 = mybir.dt.float32
TT = [128, 128, 128, 85]    # real token-tile sizes
STT = [128, 128, 128, 86]   # even-padded stationary slice sizes
TOFF = [0, 128, 256, 384]
TE = 470                    # even-padded T for matmul free dims
NKT = 8  # cin contraction tiles (1024/128)
EXP = mybir.ActivationFunctionType.Exp
LN = mybir.ActivationFunctionType.Ln
MUL = mybir.AluOpType.mult
ADD = mybir.AluOpType.add

_cache = {}


def _build():
    nc = bacc.Bacc(trn_type="TRN2", name="xattn")
    x_h = nc.dram_tensor("x", [BL, C, T], F16, kind="ExternalInput")
    wqk_h = nc.dram_tensor("wqk", [128, NKT, 2 * C], F16, kind="ExternalInput")
    wv_h = nc.dram_tensor("wv", [2, 128, NKT, 512], F16, kind="ExternalInput")
    wp_h = nc.dram_tensor("wp", [2, 128, NKT, 512], F16, kind="ExternalInput")
    bqs_h = nc.dram_tensor("bqs", [C], F32, kind="ExternalInput")
    bk_h = nc.dram_tensor("bk", [C], F32, kind="ExternalInput")
    bpe_h = nc.dram_tensor("bpe", [C], F16, kind="ExternalInput")
    kaug_h = nc.dram_tensor("kaug", [128, TE], F16, kind="ExternalInput")
    qaug_h = nc.dram_tensor("qaug", [128, TE], F16, kind="ExternalInput")
    out_h = nc.dram_tensor("out", [BL, T, C], F32, kind="ExternalOutput")

    with tile.TileContext(nc) as tc:
        with (
            tc.tile_pool(name="singles", bufs=1) as singles,
            tc.tile_pool(name="xy", bufs=2) as xy_pool,
            tc.tile_pool(name="qk", bufs=4) as qk_pool,
            tc.tile_pool(name="ep", bufs=6) as e_pool,
            tc.tile_pool(name="vw", bufs=2) as vw_pool,
            tc.tile_pool(name="pw", bufs=2) as pw_pool,
            tc.tile_pool(name="rp", bufs=2) as r_pool,
            tc.tile_pool(name="rf", bufs=2) as rf_pool,
            tc.tile_pool(name="tp", bufs=2) as t_pool,
            tc.tile_pool(name="yp", bufs=4) as y_pool,
            tc.tile_pool(name="acc", bufs=2, space="PSUM") as acc_pool,
            tc.tile_pool(name="sp", bufs=2, space="PSUM") as s_pool,
            tc.tile_pool(name="pvp", bufs=2, space="PSUM") as pv_pool,
        ):
            # ---- resident constants ----
            wqk = singles.tile([128, NKT, 2 * C], F16)
            nc.sync.dma_start(wqk[:], wqk_h[:])
            ones32 = singles.tile([128, 128], F32)
            nc.vector.memset(ones32[:], 1.0)
            ones = singles.tile([128, 128], F16)
            nc.scalar.copy(ones[:], ones32[:])
            zeros32 = singles.tile([128, NKT], F32)
            nc.vector.memset(zeros32[:], 0.0)
            onz = singles.tile([128, H, 2], F32)
            nc.vector.memset(onz[:, :, 0:1], 1.0)
            nc.vector.memset(onz[:, :, 1:2], 0.0)
            vsb = singles.tile([128, 4, H, 66], F16)
            for _tt in range(4):
                nc.scalar.copy(vsb[:, _tt, :, 64:66], onz[:])
            bqs = singles.tile([128, NKT], F32)
            nc.sync.dma_start(bqs[:], bqs_h.ap().rearrange("(cb p) -> p cb", p=128))
            bk = singles.tile([128, NKT], F32)
            nc.sync.dma_start(bk[:], bk_h.ap().rearrange("(cb p) -> p cb", p=128))
            bpe = singles.tile([1, C], F16)
            nc.sync.dma_start(bpe[:], bpe_h.ap().unsqueeze(0))
            kaug = singles.tile([128, TE], F16)
            nc.sync.dma_start(kaug[:], kaug_h[:])
            qaug = singles.tile([128, TE], F16)
            nc.sync.dma_start(qaug[:], qaug_h[:])
            # broadcast b_proj_eff across all 128 partitions (once)
            bpb = singles.tile([128, C], F32)
            for half in range(2):
                bacc_ps = acc_pool.tile([128, 512], F32, tag="acc")
                nc.tensor.matmul(
                    bacc_ps[:, 0:512],
                    lhsT=ones[0:1, 0:128],
                    rhs=bpe[0:1, 512 * half : 512 * (half + 1)],
                    start=True,
                    stop=True,
                )
                nc.vector.tensor_copy(
                    bpb[:, 512 * half : 512 * (half + 1)], bacc_ps[:, 0:512]
                )

            for b in range(BL):
                xT = xy_pool.tile([128, NKT, TE], F16, tag="xy")
                for cs in range(NKT):
                    nc.sync.dma_start(
                        xT[:, cs, 0:T], x_h[b][128 * cs : 128 * (cs + 1), :]
                    )
                nc.scalar.copy(xT[:, :, T:TE], zeros32[:].unsqueeze(2))
                yh = xy_pool.tile([128, NKT, TE], F16, tag="xy")
                nc.scalar.copy(yh[:, :, T:TE], zeros32[:].unsqueeze(2))

                pending = []  # heads with S+exp emitted, PV deferred
                state = {"pend": None}

                def norm_tail(pv, r, h, yh=yh):
                    hp, sub = h // 2, h % 2
                    bc = acc_pool.tile([128, TE], F32, tag="acc")
                    nc.tensor.matmul(
                        bc[0:64, 0:TE],
                        lhsT=ones[64:65, 0:64],
                        rhs=r[64:65, 0:TE],
                        start=True,
                        stop=True,
                    )
                    bcs = t_pool.tile([64, T], F32, tag="bcs")
                    nc.vector.tensor_copy(bcs[:, 0:T], bc[0:64, 0:T])
                    if sub == 0:
                        nc.vector.tensor_mul(
                            yh[0:64, hp, 0:T], pv[0:64, 0:T], bcs[:, 0:T]
                        )
                    else:
                        tmp = t_pool.tile([64, T], F16, tag="tmp")
                        nc.vector.tensor_mul(
                            tmp[:, 0:T], pv[0:64, 0:T], bcs[:, 0:T]
                        )
                        nc.sync.dma_start(yh[64:128, hp, 0:T], tmp[:, 0:T])

                def flush_pv():
                    h, units = pending.pop(0)
                    pv = pv_pool.tile([66, TE], F32, tag="pv")
                    for kt in range(4):
                        n = TT[kt]
                        nc.tensor.matmul(
                            pv[:, 0:TE],
                            lhsT=vsb[0:n, kt, h, 0:66],
                            rhs=units[kt // 2][0:n, kt % 2, 0:TE],
                            start=(kt == 0),
                            stop=(kt == 3),
                        )
                    # R = 1/Z = exp(-ln Z), straight off the Act engine LUTs
                    lnz = rf_pool.tile([65, TE], F32, tag="lnz")
                    nc.scalar.activation(
                        lnz[64:65, 0:TE], pv[64:65, 0:TE], LN
                    )
                    r = r_pool.tile([65, TE], F16, tag="r")
                    nc.scalar.activation(
                        r[64:65, 0:TE], lnz[64:65, 0:TE], EXP, scale=-1.0
                    )
                    if state["pend"] is not None:
                        norm_tail(*state["pend"])
                    state["pend"] = (pv, r, h)

                def emit_s(h, qsb, ksb):
                    r0 = 64 * (h % 2)
                    a0 = 64 - r0
                    units = []
                    for u in range(2):
                        s2 = s_pool.tile([128, 2, 512], F32, tag="s")
                        for j in range(2):
                            kt = 2 * u + j
                            sn = STT[kt]
                            ko = TOFF[kt]
                            nc.tensor.matmul(
                                s2[0:sn, j, 0:TE],
                                lhsT=ksb[r0 : r0 + 64, ko : ko + sn],
                                rhs=qsb[r0 : r0 + 64, 0:TE],
                                start=True,
                                stop=False,
                            )
                            nc.tensor.matmul(
                                s2[0:sn, j, 0:TE],
                                lhsT=kaug[a0 : a0 + 2, ko : ko + sn],
                                rhs=qaug[a0 : a0 + 2, 0:TE],
                                start=False,
                                stop=True,
                            )
                        e2 = e_pool.tile([128, 2, TE], F16, tag="e")
                        nc.scalar.activation(
                            e2[:, 0:2, 0:TE], s2[:, 0:2, 0:TE], EXP
                        )
                        units.append(e2)
                    pending.append((h, units))

                for c2 in range(2):
                    # V projection chunk: heads 8*c2 .. 8*c2+7
                    vw = vw_pool.tile([128, NKT, 512], F16, tag="vw")
                    nc.sync.dma_start(vw[:], wv_h[c2])
                    for tt in range(4):
                        n, sn = TT[tt], STT[tt]
                        acc = acc_pool.tile([128, 512], F32, tag="acc")
                        for kt in range(NKT):
                            nc.tensor.matmul(
                                acc[0:sn, 0:512],
                                lhsT=xT[:, kt, TOFF[tt] : TOFF[tt] + sn],
                                rhs=vw[:, kt, :],
                                start=(kt == 0),
                                stop=(kt == NKT - 1),
                            )
                        nc.vector.tensor_copy(
                            vsb[0:n, tt, 8 * c2 : 8 * c2 + 8, 0:64],
                            acc[0:n, 0:512].rearrange("p (h d) -> p h d", h=8),
                        )

                    for hp in range(4 * c2, 4 * c2 + 4):
                        # Q^T for head pair hp
                        qacc = acc_pool.tile([128, TE], F32, tag="acc")
                        for kt in range(NKT):
                            nc.tensor.matmul(
                                qacc[:, 0:TE],
                                lhsT=wqk[:, kt, 128 * hp : 128 * hp + 128],
                                rhs=xT[:, kt, 0:TE],
                                start=(kt == 0),
                                stop=(kt == NKT - 1),
                            )
                        qsb = qk_pool.tile([128, TE], F16, tag="qk")
                        nc.vector.tensor_scalar(
                            qsb[:, 0:TE], qacc[:, 0:TE],
                            0.125, bqs[:, hp : hp + 1], MUL, ADD,
                        )
                        if pending:
                            flush_pv()
                        # K^T for head pair hp
                        kacc = acc_pool.tile([128, TE], F32, tag="acc")
                        for kt in range(NKT):
                            nc.tensor.matmul(
                                kacc[:, 0:TE],
                                lhsT=wqk[:, kt, C + 128 * hp : C + 128 * hp + 128],
                                rhs=xT[:, kt, 0:TE],
                                start=(kt == 0),
                                stop=(kt == NKT - 1),
                            )
                        ksb = qk_pool.tile([128, TE], F16, tag="qk")
                        nc.vector.tensor_scalar_add(
                            ksb[:, 0:TE], kacc[:, 0:TE], bk[:, hp : hp + 1]
                        )
                        if pending:
                            flush_pv()
                        emit_s(2 * hp, qsb, ksb)
                        emit_s(2 * hp + 1, qsb, ksb)

                while pending:
                    flush_pv()
                if state["pend"] is not None:
                    norm_tail(*state["pend"])
                    state["pend"] = None

                # ---- output projection ----
                ytiles = []
                for _tt in range(4):
                    ytile = y_pool.tile([128, C], F32, tag="y", name=f"y{b}_{_tt}")
                    ytiles.append(ytile)
                for ch2 in range(2):
                    pw = pw_pool.tile([128, NKT, 512], F16, tag="pw")
                    nc.sync.dma_start(pw[:], wp_h[ch2])
                    for tt in range(4):
                        n, sn = TT[tt], STT[tt]
                        acc = acc_pool.tile([128, 512], F32, tag="acc")
                        for kt in range(NKT):
                            nc.tensor.matmul(
                                acc[0:sn, 0:512],
                                lhsT=yh[:, kt, TOFF[tt] : TOFF[tt] + sn],
                                rhs=pw[:, kt, :],
                                start=(kt == 0),
                                stop=(kt == NKT - 1),
                            )
                        nc.vector.tensor_add(
                            ytiles[tt][0:n, 512 * ch2 : 512 * (ch2 + 1)],
                            acc[0:n, 0:512],
                            bpb[0:n, 512 * ch2 : 512 * (ch2 + 1)],
                        )
                for tt in range(4):
                    n = TT[tt]
                    nc.sync.dma_start(
                        out_h[b, TOFF[tt] : TOFF[tt] + n, :], ytiles[tt][0:n, :]
                    )

    nc.compile()
    return nc


def _pack_w(w):
    # [C, n] -> [128, NKT, n] with w_packed[p, kt, j] = w[kt*128 + p, j]
    n = w.shape[1]
    return np.ascontiguousarray(
        w.reshape(NKT, 128, n).transpose(1, 0, 2), dtype=np.float16
    )


def _prep_inputs(x, W_attn, b_attn, W_proj, b_proj):
    wqk = _pack_w(np.asarray(W_attn[:, : 2 * C]))
    wv = np.stack(
        [
            _pack_w(np.asarray(W_attn[:, 2 * C + 512 * c : 2 * C + 512 * (c + 1)]))
            for c in range(2)
        ]
    )
    wpk = np.stack(
        [
            _pack_w(np.asarray(W_proj[:, 512 * c : 512 * (c + 1)]))
            for c in range(2)
        ]
    )
    bqs = (b_attn[:C].astype(np.float64) * 0.125).astype(np.float32)
    bk = np.ascontiguousarray(b_attn[C : 2 * C], dtype=np.float32)
    bv = b_attn[2 * C :].astype(np.float64)
    bpe = (b_proj.astype(np.float64) + bv @ W_proj.astype(np.float64)).astype(
        np.float16
    )
    # rank-2 additive mask: bias(k,q) = a1(q)*row0(k) + a2(q)*row1(k)
    k = np.arange(TE)
    img2 = ((k >= T1 + 1) & (k < T)).astype(np.float32)
    kzero = (k == 0).astype(np.float32)
    row0 = kzero + img2            # q in img1: mask 1 at k=0 and k in img2
    row1 = (k < T1 + 1).astype(np.float32)  # q in img2: 1 at k=0 and k in img1
    kaug = np.zeros((128, TE), dtype=np.float16)
    qaug = np.zeros((128, TE), dtype=np.float16)
    q = np.arange(TE)
    a1 = ((q >= 1) & (q < T1 + 1)).astype(np.float32)
    a2 = ((q >= T1 + 1) & (q < T)).astype(np.float32)
    for base in (0, 64):
        kaug[base + 0, :] = row0
        kaug[base + 1, :] = row1
        qaug[base + 0, :] = a1
        qaug[base + 1, :] = a2
    common = {
        "wqk": wqk, "wv": wv, "wp": wpk, "bqs": bqs, "bk": bk,
        "bpe": bpe, "kaug": kaug, "qaug": qaug,
    }
    # x -> [B, C, T] fp16 (pre-transposed so the device DMA is near-linear)
    xs = np.ascontiguousarray(
        np.asarray(x).astype(np.float16).transpose(0, 2, 1)
    )
    in_maps = []
    for cidx in range(NCORES):
        m = dict(common)
        m["x"] = np.ascontiguousarray(xs[cidx * BL : (cidx + 1) * BL])
        in_maps.append(m)
    return in_maps


def _run(x, W_attn, b_attn, W_proj, b_proj, trace=False):
    if "nc" not in _cache:
        _cache["nc"] = _build()
    nc = _cache["nc"]
    in_maps = _prep_inputs(x, W_attn, b_attn, W_proj, b_proj)
    res = run_bass_kernel_spmd(
        nc, in_maps, core_ids=list(range(NCORES)), trace=trace
    )
    out = np.concatenate([r["out"] for r in res.results], axis=0)
    return out.astype(np.float32), res


def kernel(x, W_attn, b_attn, W_proj, b_proj):
    out, _ = _run(x, W_attn, b_attn, W_proj, b_proj, trace=False)
    return out


# revision 11
# speedup vs baseline: 1.8864x; 1.5229x over previous
"""CrossOnlyAttention Trainium2 kernel (v5).

Data-parallel over batch: 64 batches -> 8 cores x 8 batches. fp16 matmuls.

Per-core dataflow (per batch):
  x^T [C,T] in SBUF -> QKV projections:
     Q^T,K^T head-major [2 heads x 64, T] per head-pair (bias + 1/8 scale fused
     into the PSUM->SBUF eviction on DVE), V token-major [T, 64] per head with
     a ones column appended (V_aug) so the PV matmul also produces the softmax
     denominator Z as row 64.
  Scores S^T[k,q] = K^T.T @ Q^T (contraction=64 on PE rows 0:64 / 64:128 per
     head parity). The two heads of a pair write the SAME key-tile into the
     two banks of one 2-bank PSUM unit, so each biased-exp activation covers
     both heads at once through a 3D access pattern — the additive cross-mask
     reduces (up to a softmax-cancelled per-query constant) to a per-KEY bias
     selected by the query's image, identical for both heads, applied as the
     activation's per-partition bias operand over the two query-image free
     slices ([1:235] vs [235:470]; q=0 gets no bias).
  PV: Y^T[66,q] = V_aug.T @ E accumulated over 4 k-tiles; row 64 = Z.
  Normalize: Z row copied to SBUF by the Act engine (plain Copy, lives in
     every LUT set -> no table reload), R=1/Z via the DVE fast reciprocal
     (custom op; SBUF-only) + f16 cast, broadcast R across 64 partitions with
     a K=1 matmul into an acc-pool bank, copy to SBUF, DVE multiply ->
     Yhat^T [C,T]; odd heads shifted to partitions 64:128 via SBUF->SBUF DMA.
     V-bias is folded into b_proj on the host.
  Proj: out[t, c] accumulated over 8 cin tiles at N=512; b_proj added during
     the DVE eviction from a broadcast bias tile (built once by K=1 matmuls).

Emission is software-pipelined across head-pairs so dependency-gated matmuls
sit ~1.6us behind their producers and the PE never idles long enough for the
HAM clock gate to re-throttle:
  iteration hp: QKq(hp) | S-units kt2,kt3(hp-1) | QKk(hp) | PV(h0 of hp-1) |
                S-units kt0,kt1(hp) | PV(h1 of hp-1), norm tails one head late.

fp16 ISA restrictions (walrus s3d3_mm): every matmul operand free-dim count
and the PSUM dst free count must be EVEN, dst start_partition must be 0.
Token free dims padded 469->470 (TE), ragged stationary slices 85->86 (STT).
"""

import os
import sys

import numpy as np

for _p in (
    "/opt/trn_rl_repo",
    "/root/.axon_site",
    "/root/.axon_site/_ro/trn_rl_repo",
    "/root/.axon_site/_ro/pypackages",
):
    if os.path.isdir(_p) and _p not in sys.path:
        sys.path.append(_p)

import concourse.bass as bass  # noqa: E402,F401
import concourse.tile as tile  # noqa: E402
from concourse import bacc, mybir  # noqa: E402
from concourse.bass_utils import run_bass_kernel_spmd  # noqa: E402

B, T, C = 64, 469, 1024
H, HD = 16, 64
T1 = 234
NCORES = 8
BL = B // NCORES
F16 = mybir.dt.float16
F32 = mybir.dt.float32
TT = [128, 128, 128, 85]    # real token-tile sizes
STT = [128, 128, 128, 86]   # even-padded stationary slice sizes
TOFF = [0, 128, 256, 384]
TE = 470                    # even-padded T for matmul free dims
NKT = 8  # cin contraction tiles (1024/128)
EXP = mybir.ActivationFunctionType.Exp
MUL = mybir.AluOpType.mult
ADD = mybir.AluOpType.add

_cache = {}


def _build():
    nc = bacc.Bacc(trn_type="TRN2", name="xattn")
    x_h = nc.dram_tensor("x", [BL, C, T], F16, kind="ExternalInput")
    wqk_h = nc.dram_tensor("wqk", [128, NKT, 2 * C], F16, kind="ExternalInput")
    wv_h = nc.dram_tensor("wv", [2, 128, NKT, 512], F16, kind="ExternalInput")
    wp_h = nc.dram_tensor("wp", [2, 128, NKT, 512], F16, kind="ExternalInput")
    bqs_h = nc.dram_tensor("bqs", [C], F32, kind="ExternalInput")
    bk_h = nc.dram_tensor("bk", [C], F32, kind="ExternalInput")
    bpe_h = nc.dram_tensor("bpe", [C], F16, kind="ExternalInput")
    mb_h = nc.dram_tensor("mb", [128, 4, 2], F32, kind="ExternalInput")
    out_h = nc.dram_tensor("out", [BL, T, C], F32, kind="ExternalOutput")

    with tile.TileContext(nc) as tc:
        with (
            tc.tile_pool(name="singles", bufs=1) as singles,
            tc.tile_pool(name="xy", bufs=2) as xy_pool,
            tc.tile_pool(name="qk", bufs=4) as qk_pool,
            tc.tile_pool(name="ep", bufs=9) as e_pool,
            tc.tile_pool(name="vw", bufs=2) as vw_pool,
            tc.tile_pool(name="pw", bufs=2) as pw_pool,
            tc.tile_pool(name="rp", bufs=2) as r_pool,
            tc.tile_pool(name="rf", bufs=2) as rf_pool,
            tc.tile_pool(name="tp", bufs=2) as t_pool,
            tc.tile_pool(name="yp", bufs=4) as y_pool,
            tc.tile_pool(name="acc", bufs=2, space="PSUM") as acc_pool,
            tc.tile_pool(name="sp", bufs=2, space="PSUM") as s_pool,
            tc.tile_pool(name="pvp", bufs=2, space="PSUM") as pv_pool,
        ):
            # ---- resident constants ----
            wqk = singles.tile([128, NKT, 2 * C], F16)
            nc.sync.dma_start(wqk[:], wqk_h[:])
            ones32 = singles.tile([128, 128], F32)
            nc.vector.memset(ones32[:], 1.0)
            ones = singles.tile([128, 128], F16)
            nc.scalar.copy(ones[:], ones32[:])
            zeros32 = singles.tile([128, NKT], F32)
            nc.vector.memset(zeros32[:], 0.0)
            onz = singles.tile([128, H, 2], F32)
            nc.vector.memset(onz[:, :, 0:1], 1.0)
            nc.vector.memset(onz[:, :, 1:2], 0.0)
            vsb = singles.tile([128, 4, H, 66], F16)
            for _tt in range(4):
                nc.scalar.copy(vsb[:, _tt, :, 64:66], onz[:])
            bqs = singles.tile([128, NKT], F32)
            nc.sync.dma_start(bqs[:], bqs_h.ap().rearrange("(cb p) -> p cb", p=128))
            bk = singles.tile([128, NKT], F32)
            nc.sync.dma_start(bk[:], bk_h.ap().rearrange("(cb p) -> p cb", p=128))
            bpe = singles.tile([1, C], F16)
            nc.sync.dma_start(bpe[:], bpe_h.ap().unsqueeze(0))
            mb = singles.tile([128, 4, 2], F32)
            nc.sync.dma_start(mb[:], mb_h[:])
            # broadcast b_proj_eff across all 128 partitions (once)
            bpb = singles.tile([128, C], F32)
            for half in range(2):
                bacc_ps = acc_pool.tile([128, 512], F32, tag="acc")
                nc.tensor.matmul(
                    bacc_ps[:, 0:512],
                    lhsT=ones[0:1, 0:128],
                    rhs=bpe[0:1, 512 * half : 512 * (half + 1)],
                    start=True,
                    stop=True,
                )
                nc.vector.tensor_copy(
                    bpb[:, 512 * half : 512 * (half + 1)], bacc_ps[:, 0:512]
                )

            for b in range(BL):
                xT = xy_pool.tile([128, NKT, TE], F16, tag="xy")
                for cs in range(NKT):
                    nc.sync.dma_start(
                        xT[:, cs, 0:T], x_h[b][128 * cs : 128 * (cs + 1), :]
                    )
                nc.scalar.copy(xT[:, :, T:TE], zeros32[:].unsqueeze(2))
                yh = xy_pool.tile([128, NKT, TE], F16, tag="xy")
                nc.scalar.copy(yh[:, :, T:TE], zeros32[:].unsqueeze(2))

                # pipeline state
                units = {}        # hp -> [e2 unit per kt]
                halfpend = []     # [(hp, qsb, ksb)] with kt2/kt3 not emitted
                pvq = []          # heads ready for PV flush
                state = {"pend": None}

                def norm_tail(pv, zr, h, yh=yh):
                    # broadcast Z across 64 partitions, then 1/Z on the
                    # base-0 broadcast (reciprocal_approx_fast silently
                    # corrupts on offset-base single-partition APs)
                    hp, sub = h // 2, h % 2
                    bc = acc_pool.tile([128, TE], F32, tag="acc")
                    nc.tensor.matmul(
                        bc[0:64, 0:TE],
                        lhsT=ones[64:65, 0:64],
                        rhs=zr[64:65, 0:TE],
                        start=True,
                        stop=True,
                    )
                    bcz = t_pool.tile([64, T], F32, tag="bcz")
                    nc.vector.tensor_copy(bcz[:, 0:T], bc[0:64, 0:T])
                    bcs = t_pool.tile([64, T], F32, tag="bcs")
                    nc.vector.reciprocal_approx_fast(bcs[:, 0:T], bcz[:, 0:T])
                    if sub == 0:
                        nc.vector.tensor_mul(
                            yh[0:64, hp, 0:T], pv[0:64, 0:T], bcs[:, 0:T]
                        )
                    else:
                        tmp = t_pool.tile([64, T], F16, tag="tmp")
                        nc.vector.tensor_mul(
                            tmp[:, 0:T], pv[0:64, 0:T], bcs[:, 0:T]
                        )
                        nc.sync.dma_start(yh[64:128, hp, 0:T], tmp[:, 0:T])

                def emit_units(hp, qsb, ksb, kts):
                    # one 2-bank PSUM unit per key-tile, holding BOTH heads of
                    # the pair; 3 biased-exp activations cover both heads
                    for kt in kts:
                        n, sn, ko = TT[kt], STT[kt], TOFF[kt]
                        s2 = s_pool.tile([128, 2, 512], F32, tag="s")
                        for j in range(2):
                            r0 = 64 * j
                            nc.tensor.matmul(
                                s2[0:sn, j, 0:TE],
                                lhsT=ksb[r0 : r0 + 64, ko : ko + sn],
                                rhs=qsb[r0 : r0 + 64, 0:TE],
                                start=True,
                                stop=True,
                            )
                        e2 = e_pool.tile([128, 2, TE], F16, tag="e")
                        nc.scalar.activation(
                            e2[0:n, 0:2, 0:1], s2[0:n, 0:2, 0:1], EXP
                        )
                        nc.scalar.activation(
                            e2[0:n, 0:2, 1 : T1 + 1],
                            s2[0:n, 0:2, 1 : T1 + 1],
                            EXP,
                            bias=mb[0:n, kt, 0:1],
                        )
                        nc.scalar.activation(
                            e2[0:n, 0:2, T1 + 1 : TE],
                            s2[0:n, 0:2, T1 + 1 : TE],
                            EXP,
                            bias=mb[0:n, kt, 1:2],
                        )
                        units[hp].append(e2)

                def flush_pv():
                    h = pvq.pop(0)
                    hp, j = h // 2, h % 2
                    pv = pv_pool.tile([66, TE], F32, tag="pv")
                    for kt in range(4):
                        n = TT[kt]
                        nc.tensor.matmul(
                            pv[:, 0:TE],
                            lhsT=vsb[0:n, kt, h, 0:66],
                            rhs=units[hp][kt][0:n, j, 0:TE],
                            start=(kt == 0),
                            stop=(kt == 3),
                        )
                    if j == 1:
                        del units[hp]
                    # Z row -> SBUF f16 on the Act engine (Copy lives in
                    # every LUT set -> no table reload)
                    zr = r_pool.tile([65, TE], F16, tag="r")
                    nc.scalar.copy(zr[64:65, 0:TE], pv[64:65, 0:TE])
                    if state["pend"] is not None:
                        norm_tail(*state["pend"])
                    state["pend"] = (pv, zr, h)

                for c2 in range(2):
                    # V projection chunk: heads 8*c2 .. 8*c2+7
                    vw = vw_pool.tile([128, NKT, 512], F16, tag="vw")
                    nc.sync.dma_start(vw[:], wv_h[c2])
                    for tt in range(4):
                        n, sn = TT[tt], STT[tt]
                        acc = acc_pool.tile([128, 512], F32, tag="acc")
                        for kt in range(NKT):
                            nc.tensor.matmul(
                                acc[0:sn, 0:512],
                                lhsT=xT[:, kt, TOFF[tt] : TOFF[tt] + sn],
                                rhs=vw[:, kt, :],
                                start=(kt == 0),
                                stop=(kt == NKT - 1),
                            )
                        nc.vector.tensor_copy(
                            vsb[0:n, tt, 8 * c2 : 8 * c2 + 8, 0:64],
                            acc[0:n, 0:512].rearrange("p (h d) -> p h d", h=8),
                        )

                    for hp in range(4 * c2, 4 * c2 + 4):
                        # Q^T for head pair hp
                        qacc = acc_pool.tile([128, TE], F32, tag="acc")
                        for kt in range(NKT):
                            nc.tensor.matmul(
                                qacc[:, 0:TE],
                                lhsT=wqk[:, kt, 128 * hp : 128 * hp + 128],
                                rhs=xT[:, kt, 0:TE],
                                start=(kt == 0),
                                stop=(kt == NKT - 1),
                            )
                        qsb = qk_pool.tile([128, TE], F16, tag="qk")
                        nc.vector.tensor_scalar(
                            qsb[:, 0:TE], qacc[:, 0:TE],
                            0.125, bqs[:, hp : hp + 1], MUL, ADD,
                        )
                        if halfpend:
                            php, pq, pk = halfpend.pop(0)
                            emit_units(php, pq, pk, (2, 3))
                            pvq.append(2 * php)
                            pvq.append(2 * php + 1)
                        # K^T for head pair hp
                        kacc = acc_pool.tile([128, TE], F32, tag="acc")
                        for kt in range(NKT):
                            nc.tensor.matmul(
                                kacc[:, 0:TE],
                                lhsT=wqk[:, kt, C + 128 * hp : C + 128 * hp + 128],
                                rhs=xT[:, kt, 0:TE],
                                start=(kt == 0),
                                stop=(kt == NKT - 1),
                            )
                        ksb = qk_pool.tile([128, TE], F16, tag="qk")
                        nc.vector.tensor_scalar_add(
                            ksb[:, 0:TE], kacc[:, 0:TE], bk[:, hp : hp + 1]
                        )
                        if pvq:
                            flush_pv()
                        units[hp] = []
                        emit_units(hp, qsb, ksb, (0, 1))
                        halfpend.append((hp, qsb, ksb))
                        if pvq:
                            flush_pv()

                # drain the pipeline
                while halfpend:
                    php, pq, pk = halfpend.pop(0)
                    emit_units(php, pq, pk, (2, 3))
                    pvq.append(2 * php)
                    pvq.append(2 * php + 1)
                while pvq:
                    flush_pv()
                if state["pend"] is not None:
                    norm_tail(*state["pend"])
                    state["pend"] = None

                # ---- output projection ----
                ytiles = []
                for _tt in range(4):
                    ytile = y_pool.tile([128, C], F32, tag="y", name=f"y{b}_{_tt}")
                    ytiles.append(ytile)
                for ch2 in range(2):
                    pw = pw_pool.tile([128, NKT, 512], F16, tag="pw")
                    nc.sync.dma_start(pw[:], wp_h[ch2])
                    for tt in range(4):
                        n, sn = TT[tt], STT[tt]
                        acc = acc_pool.tile([128, 512], F32, tag="acc")
                        for kt in range(NKT):
                            nc.tensor.matmul(
                                acc[0:sn, 0:512],
                                lhsT=yh[:, kt, TOFF[tt] : TOFF[tt] + sn],
                                rhs=pw[:, kt, :],
                                start=(kt == 0),
                                stop=(kt == NKT - 1),
                            )
                        nc.vector.tensor_add(
                            ytiles[tt][0:n, 512 * ch2 : 512 * (ch2 + 1)],
                            acc[0:n, 0:512],
                            bpb[0:n, 512 * ch2 : 512 * (ch2 + 1)],
                        )
                for tt in range(4):
                    n = TT[tt]
                    nc.sync.dma_start(
                        out_h[b, TOFF[tt] : TOFF[tt] + n, :], ytiles[tt][0:n, :]
                    )

    nc.compile()
    return nc


def _pack_w(w):
    # [C, n] -> [128, NKT, n] with w_packed[p, kt, j] = w[kt*128 + p, j]
    n = w.shape[1]
    return np.ascontiguousarray(
        w.reshape(NKT, 128, n).transpose(1, 0, 2), dtype=np.float16
    )


def _prep_inputs(x, W_attn, b_attn, W_proj, b_proj):
    wqk = _pack_w(np.asarray(W_attn[:, : 2 * C]))
    wv = np.stack(
        [
            _pack_w(np.asarray(W_attn[:, 2 * C + 512 * c : 2 * C + 512 * (c + 1)]))
            for c in range(2)
        ]
    )
    wpk = np.stack(
        [
            _pack_w(np.asarray(W_proj[:, 512 * c : 512 * (c + 1)]))
            for c in range(2)
        ]
    )
    bqs = (b_attn[:C].astype(np.float64) * 0.125).astype(np.float32)
    bk = np.ascontiguousarray(b_attn[C : 2 * C], dtype=np.float32)
    bv = b_attn[2 * C :].astype(np.float64)
    bpe = (b_proj.astype(np.float64) + bv @ W_proj.astype(np.float64)).astype(
        np.float16
    )
    # mask bias per key position: col 0 -> query in image1, col 1 -> image2
    mbm = np.zeros((2, 512), dtype=np.float32)
    k = np.arange(T)
    img2 = (k >= T1 + 1).astype(np.float32)
    kzero = (k == 0).astype(np.float32)
    mbm[0, :T] = kzero + img2          # q in img1: mask 1 at k=0 and k in img2
    mbm[1, :T] = 1.0 - img2            # q in img2: mask 1 at k=0 and k in img1
    # device layout [p, kt, j]: mb_dev[p, kt, j] = mbm[j, kt*128 + p]
    mb_dev = np.ascontiguousarray(mbm.reshape(2, 4, 128).transpose(2, 1, 0))
    common = {
        "wqk": wqk, "wv": wv, "wp": wpk, "bqs": bqs, "bk": bk,
        "bpe": bpe, "mb": mb_dev,
    }
    # x -> [B, C, T] fp16 (pre-transposed so the device DMA is near-linear)
    xs = np.ascontiguousarray(
        np.asarray(x).astype(np.float16).transpose(0, 2, 1)
    )
    in_maps = []
    for cidx in range(NCORES):
        m = dict(common)
        m["x"] = np.ascontiguousarray(xs[cidx * BL : (cidx + 1) * BL])
        in_maps.append(m)
    return in_maps


def _run(x, W_attn, b_attn, W_proj, b_proj, trace=False):
    if "nc" not in _cache:
        _cache["nc"] = _build()
    nc = _cache["nc"]
    in_maps = _prep_inputs(x, W_attn, b_attn, W_proj, b_proj)
    res = run_bass_kernel_spmd(
        nc, in_maps, core_ids=list(range(NCORES)), trace=trace
    )
    out = np.concatenate([r["out"] for r in res.results], axis=0)
    return out.astype(np.float32), res


def kernel(x, W_attn, b_attn, W_proj, b_proj):
    out, _ = _run(x, W_attn, b_attn, W_proj, b_proj, trace=False)
    return out


# revision 12
# speedup vs baseline: 2.0570x; 1.0904x over previous
"""CrossOnlyAttention Trainium2 kernel (v6).

Data-parallel over batch: 64 batches -> 8 cores x 8 batches. fp16 matmuls.

Per-core dataflow (per batch):
  x^T [C,T] in SBUF -> QKV projections:
     Q^T,K^T head-major [2 heads x 64, T] per head-pair (bias + 1/8 scale fused
     into the PSUM->SBUF eviction on DVE), V token-major [T, 64] per head with
     a ones column appended (V_aug) so the PV matmul also produces the softmax
     denominator Z as row 64.
  Scores S^T[k,q] = K^T.T @ Q^T (contraction=64 on PE rows 0:64 / 64:128 per
     head parity). The two heads of a pair write the SAME key-tile into the
     two banks of one 2-bank PSUM unit, so each biased-exp activation covers
     both heads at once through a 3D access pattern — the additive cross-mask
     reduces (up to a softmax-cancelled per-query constant) to a per-KEY bias
     selected by the query's image, identical for both heads, applied as the
     activation's per-partition bias operand over the two query-image free
     slices ([1:235] vs [235:470]; q=0 gets no bias).
  PV: Y^T[66,q] = V_aug.T @ E accumulated over 4 k-tiles; row 64 = Z.
  Normalize: Z row -> SBUF f16 (DVE), broadcast Z across 64 partitions with a
     K=1 matmul, copy to SBUF, R=1/Z via the DVE fast reciprocal on the
     base-0 broadcast (the custom op corrupts on offset-base APs), DVE
     multiply -> Yhat^T [C,T]; odd heads shifted to partitions 64:128 via
     SBUF->SBUF DMA. V-bias is folded into b_proj on the host.
  Proj: out[t, c] accumulated over 8 cin tiles at N=512; b_proj added during
     the DVE eviction from a broadcast bias tile (built once by K=1 matmuls).

Emission is software-pipelined across head-pairs AND batches:
  iteration (b,hp): QKq | S-units kt2,kt3(prev pair) | QKk | PV flush |
                    [proj of batch b-1 when hp==1] | S-units kt0,kt1 |
                    PV flush,
with norm tails one head late and each batch's projection hoisted into the
next batch's pipeline, so dependency-gated matmuls always sit well behind
their producers and the PE never idles long enough for the HAM clock gate to
re-throttle.

fp16 ISA restrictions (walrus s3d3_mm): every matmul operand free-dim count
and the PSUM dst free count must be EVEN, dst start_partition must be 0.
Token free dims padded 469->470 (TE), ragged stationary slices 85->86 (STT).
"""

import os
import sys

import numpy as np

for _p in (
    "/opt/trn_rl_repo",
    "/root/.axon_site",
    "/root/.axon_site/_ro/trn_rl_repo",
    "/root/.axon_site/_ro/pypackages",
):
    if os.path.isdir(_p) and _p not in sys.path:
        sys.path.append(_p)

import concourse.bass as bass  # noqa: E402,F401
import concourse.tile as tile  # noqa: E402
from concourse import bacc, mybir  # noqa: E402
from concourse.bass_utils import run_bass_kernel_spmd  # noqa: E402

B, T, C = 64, 469, 1024
H, HD = 16, 64
T1 = 234
NCORES = 8
BL = B // NCORES
F16 = mybir.dt.float16
F32 = mybir.dt.float32
TT = [128, 128, 128, 85]    # real token-tile sizes
STT = [128, 128, 128, 86]   # even-padded stationary slice sizes
TOFF = [0, 128, 256, 384]
TE = 470                    # even-padded T for matmul free dims
NKT = 8  # cin contraction tiles (1024/128)
EXP = mybir.ActivationFunctionType.Exp
MUL = mybir.AluOpType.mult
ADD = mybir.AluOpType.add

_cache = {}


def _build():
    nc = bacc.Bacc(trn_type="TRN2", name="xattn")
    x_h = nc.dram_tensor("x", [BL, C, T], F16, kind="ExternalInput")
    wqk_h = nc.dram_tensor("wqk", [128, NKT, 2 * C], F16, kind="ExternalInput")
    wv_h = nc.dram_tensor("wv", [2, 128, NKT, 512], F16, kind="ExternalInput")
    wp_h = nc.dram_tensor("wp", [2, 128, NKT, 512], F16, kind="ExternalInput")
    bqs_h = nc.dram_tensor("bqs", [C], F32, kind="ExternalInput")
    bk_h = nc.dram_tensor("bk", [C], F32, kind="ExternalInput")
    bpe_h = nc.dram_tensor("bpe", [C], F16, kind="ExternalInput")
    mb_h = nc.dram_tensor("mb", [128, 4, 2], F32, kind="ExternalInput")
    out_h = nc.dram_tensor("out", [BL, T, C], F32, kind="ExternalOutput")

    with tile.TileContext(nc) as tc:
        with (
            tc.tile_pool(name="singles", bufs=1) as singles,
            tc.tile_pool(name="xy", bufs=4) as xy_pool,
            tc.tile_pool(name="qk", bufs=4) as qk_pool,
            tc.tile_pool(name="ep", bufs=9) as e_pool,
            tc.tile_pool(name="vw", bufs=2) as vw_pool,
            tc.tile_pool(name="pw", bufs=2) as pw_pool,
            tc.tile_pool(name="rp", bufs=2) as r_pool,
            tc.tile_pool(name="tp", bufs=2) as t_pool,
            tc.tile_pool(name="yp", bufs=4) as y_pool,
            tc.tile_pool(name="acc", bufs=2, space="PSUM") as acc_pool,
            tc.tile_pool(name="sp", bufs=2, space="PSUM") as s_pool,
            tc.tile_pool(name="pvp", bufs=2, space="PSUM") as pv_pool,
        ):
            # ---- resident constants ----
            wqk = singles.tile([128, NKT, 2 * C], F16)
            nc.sync.dma_start(wqk[:], wqk_h[:])
            ones32 = singles.tile([128, 128], F32)
            nc.vector.memset(ones32[:], 1.0)
            ones = singles.tile([128, 128], F16)
            nc.scalar.copy(ones[:], ones32[:])
            zeros32 = singles.tile([128, NKT], F32)
            nc.vector.memset(zeros32[:], 0.0)
            onz = singles.tile([128, H, 2], F32)
            nc.vector.memset(onz[:, :, 0:1], 1.0)
            nc.vector.memset(onz[:, :, 1:2], 0.0)
            vsb = singles.tile([128, 4, H, 66], F16)
            for _tt in range(4):
                nc.scalar.copy(vsb[:, _tt, :, 64:66], onz[:])
            bqs = singles.tile([128, NKT], F32)
            nc.sync.dma_start(bqs[:], bqs_h.ap().rearrange("(cb p) -> p cb", p=128))
            bk = singles.tile([128, NKT], F32)
            nc.sync.dma_start(bk[:], bk_h.ap().rearrange("(cb p) -> p cb", p=128))
            bpe = singles.tile([1, C], F16)
            nc.sync.dma_start(bpe[:], bpe_h.ap().unsqueeze(0))
            mb = singles.tile([128, 4, 2], F32)
            nc.sync.dma_start(mb[:], mb_h[:])
            # broadcast b_proj_eff across all 128 partitions (once)
            bpb = singles.tile([128, C], F32)
            for half in range(2):
                bacc_ps = acc_pool.tile([128, 512], F32, tag="acc")
                nc.tensor.matmul(
                    bacc_ps[:, 0:512],
                    lhsT=ones[0:1, 0:128],
                    rhs=bpe[0:1, 512 * half : 512 * (half + 1)],
                    start=True,
                    stop=True,
                )
                nc.vector.tensor_copy(
                    bpb[:, 512 * half : 512 * (half + 1)], bacc_ps[:, 0:512]
                )

            # ---- cross-batch pipeline state ----
            units = {}        # ghp -> [e2 unit per kt]
            halfpend = []     # [(ghp, qsb, ksb)] with kt2/kt3 not emitted
            pvq = []          # [(ghp, j, yh)] heads ready for PV flush
            projq = []        # [(b, yh)] batches awaiting projection
            state = {"pend": None}

            def norm_tail(pv, zr, h, yh):
                hps = (h % 16) // 2
                sub = h % 2
                bc = acc_pool.tile([128, TE], F32, tag="acc")
                nc.tensor.matmul(
                    bc[0:64, 0:TE],
                    lhsT=ones[64:65, 0:64],
                    rhs=zr[64:65, 0:TE],
                    start=True,
                    stop=True,
                )
                bcz = t_pool.tile([64, T], F32, tag="bcz")
                nc.vector.tensor_copy(bcz[:, 0:T], bc[0:64, 0:T])
                bcs = t_pool.tile([64, T], F32, tag="bcs")
                nc.vector.reciprocal_approx_fast(bcs[:, 0:T], bcz[:, 0:T])
                if sub == 0:
                    nc.vector.tensor_mul(
                        yh[0:64, hps, 0:T], pv[0:64, 0:T], bcs[:, 0:T]
                    )
                else:
                    tmp = t_pool.tile([64, T], F16, tag="tmp")
                    nc.vector.tensor_mul(
                        tmp[:, 0:T], pv[0:64, 0:T], bcs[:, 0:T]
                    )
                    nc.sync.dma_start(yh[64:128, hps, 0:T], tmp[:, 0:T])

            def emit_units(ghp, qsb, ksb, kts):
                # one 2-bank PSUM unit per key-tile, holding BOTH heads of
                # the pair; 3 biased-exp activations cover both heads
                for kt in kts:
                    n, sn, ko = TT[kt], STT[kt], TOFF[kt]
                    s2 = s_pool.tile([128, 2, 512], F32, tag="s")
                    for j in range(2):
                        r0 = 64 * j
                        nc.tensor.matmul(
                            s2[0:sn, j, 0:TE],
                            lhsT=ksb[r0 : r0 + 64, ko : ko + sn],
                            rhs=qsb[r0 : r0 + 64, 0:TE],
                            start=True,
                            stop=True,
                        )
                    e2 = e_pool.tile([128, 2, TE], F16, tag="e")
                    nc.scalar.activation(
                        e2[0:n, 0:2, 0:1], s2[0:n, 0:2, 0:1], EXP
                    )
                    nc.scalar.activation(
                        e2[0:n, 0:2, 1 : T1 + 1],
                        s2[0:n, 0:2, 1 : T1 + 1],
                        EXP,
                        bias=mb[0:n, kt, 0:1],
                    )
                    nc.scalar.activation(
                        e2[0:n, 0:2, T1 + 1 : TE],
                        s2[0:n, 0:2, T1 + 1 : TE],
                        EXP,
                        bias=mb[0:n, kt, 1:2],
                    )
                    units[ghp].append(e2)

            def flush_pv():
                ghp, j, yh = pvq.pop(0)
                hh = (ghp % 8) * 2 + j   # head index within the batch
                pv = pv_pool.tile([66, TE], F32, tag="pv")
                for kt in range(4):
                    n = TT[kt]
                    nc.tensor.matmul(
                        pv[:, 0:TE],
                        lhsT=vsb[0:n, kt, hh, 0:66],
                        rhs=units[ghp][kt][0:n, j, 0:TE],
                        start=(kt == 0),
                        stop=(kt == 3),
                    )
                if j == 1:
                    del units[ghp]
                # Z row -> SBUF f16 for the broadcast matmul
                zr = r_pool.tile([65, TE], F16, tag="r")
                nc.vector.tensor_copy(zr[64:65, 0:TE], pv[64:65, 0:TE])
                if state["pend"] is not None:
                    norm_tail(*state["pend"])
                state["pend"] = (pv, zr, hh, yh)

            def pop_halfpend():
                pghp, pq, pk, pyh = halfpend.pop(0)
                emit_units(pghp, pq, pk, (2, 3))
                pvq.append((pghp, 0, pyh))
                pvq.append((pghp, 1, pyh))

            def emit_proj(pb, yh):
                ytiles = []
                for _tt in range(4):
                    ytile = y_pool.tile(
                        [128, C], F32, tag="y", name=f"y{pb}_{_tt}"
                    )
                    ytiles.append(ytile)
                for ch2 in range(2):
                    pw = pw_pool.tile([128, NKT, 512], F16, tag="pw")
                    nc.sync.dma_start(pw[:], wp_h[ch2])
                    for tt in range(4):
                        n, sn = TT[tt], STT[tt]
                        acc = acc_pool.tile([128, 512], F32, tag="acc")
                        for kt in range(NKT):
                            nc.tensor.matmul(
                                acc[0:sn, 0:512],
                                lhsT=yh[:, kt, TOFF[tt] : TOFF[tt] + sn],
                                rhs=pw[:, kt, :],
                                start=(kt == 0),
                                stop=(kt == NKT - 1),
                            )
                        nc.vector.tensor_add(
                            ytiles[tt][0:n, 512 * ch2 : 512 * (ch2 + 1)],
                            acc[0:n, 0:512],
                            bpb[0:n, 512 * ch2 : 512 * (ch2 + 1)],
                        )
                for tt in range(4):
                    n = TT[tt]
                    nc.sync.dma_start(
                        out_h[pb, TOFF[tt] : TOFF[tt] + n, :], ytiles[tt][0:n, :]
                    )

            for b in range(BL):
                xT = xy_pool.tile([128, NKT, TE], F16, tag="xy")
                for cs in range(NKT):
                    nc.sync.dma_start(
                        xT[:, cs, 0:T], x_h[b][128 * cs : 128 * (cs + 1), :]
                    )
                nc.scalar.copy(xT[:, :, T:TE], zeros32[:].unsqueeze(2))
                yh = xy_pool.tile([128, NKT, TE], F16, tag="xy")
                nc.scalar.copy(yh[:, :, T:TE], zeros32[:].unsqueeze(2))

                for c2 in range(2):
                    # V projection chunk: heads 8*c2 .. 8*c2+7
                    vw = vw_pool.tile([128, NKT, 512], F16, tag="vw")
                    nc.sync.dma_start(vw[:], wv_h[c2])
                    for tt in range(4):
                        n, sn = TT[tt], STT[tt]
                        acc = acc_pool.tile([128, 512], F32, tag="acc")
                        for kt in range(NKT):
                            nc.tensor.matmul(
                                acc[0:sn, 0:512],
                                lhsT=xT[:, kt, TOFF[tt] : TOFF[tt] + sn],
                                rhs=vw[:, kt, :],
                                start=(kt == 0),
                                stop=(kt == NKT - 1),
                            )
                        nc.vector.tensor_copy(
                            vsb[0:n, tt, 8 * c2 : 8 * c2 + 8, 0:64],
                            acc[0:n, 0:512].rearrange("p (h d) -> p h d", h=8),
                        )

                    for hp in range(4 * c2, 4 * c2 + 4):
                        ghp = 8 * b + hp
                        # Q^T for head pair hp
                        qacc = acc_pool.tile([128, TE], F32, tag="acc")
                        for kt in range(NKT):
                            nc.tensor.matmul(
                                qacc[:, 0:TE],
                                lhsT=wqk[:, kt, 128 * hp : 128 * hp + 128],
                                rhs=xT[:, kt, 0:TE],
                                start=(kt == 0),
                                stop=(kt == NKT - 1),
                            )
                        qsb = qk_pool.tile([128, TE], F16, tag="qk")
                        nc.vector.tensor_scalar(
                            qsb[:, 0:TE], qacc[:, 0:TE],
                            0.125, bqs[:, hp : hp + 1], MUL, ADD,
                        )
                        if halfpend:
                            pop_halfpend()
                        # K^T for head pair hp
                        kacc = acc_pool.tile([128, TE], F32, tag="acc")
                        for kt in range(NKT):
                            nc.tensor.matmul(
                                kacc[:, 0:TE],
                                lhsT=wqk[:, kt, C + 128 * hp : C + 128 * hp + 128],
                                rhs=xT[:, kt, 0:TE],
                                start=(kt == 0),
                                stop=(kt == NKT - 1),
                            )
                        ksb = qk_pool.tile([128, TE], F16, tag="qk")
                        nc.vector.tensor_scalar_add(
                            ksb[:, 0:TE], kacc[:, 0:TE], bk[:, hp : hp + 1]
                        )
                        if pvq:
                            flush_pv()
                        if hp == 1 and projq:
                            emit_proj(*projq.pop(0))
                        units[ghp] = []
                        emit_units(ghp, qsb, ksb, (0, 1))
                        halfpend.append((ghp, qsb, ksb, yh))
                        if pvq:
                            flush_pv()

                projq.append((b, yh))

            # ---- drain the pipeline ----
            while halfpend:
                pop_halfpend()
            while pvq:
                flush_pv()
            if state["pend"] is not None:
                norm_tail(*state["pend"])
                state["pend"] = None
            while projq:
                emit_proj(*projq.pop(0))

    nc.compile()
    return nc


def _pack_w(w):
    # [C, n] -> [128, NKT, n] with w_packed[p, kt, j] = w[kt*128 + p, j]
    n = w.shape[1]
    return np.ascontiguousarray(
        w.reshape(NKT, 128, n).transpose(1, 0, 2), dtype=np.float16
    )


def _prep_inputs(x, W_attn, b_attn, W_proj, b_proj):
    wqk = _pack_w(np.asarray(W_attn[:, : 2 * C]))
    wv = np.stack(
        [
            _pack_w(np.asarray(W_attn[:, 2 * C + 512 * c : 2 * C + 512 * (c + 1)]))
            for c in range(2)
        ]
    )
    wpk = np.stack(
        [
            _pack_w(np.asarray(W_proj[:, 512 * c : 512 * (c + 1)]))
            for c in range(2)
        ]
    )
    bqs = (b_attn[:C].astype(np.float64) * 0.125).astype(np.float32)
    bk = np.ascontiguousarray(b_attn[C : 2 * C], dtype=np.float32)
    bv = b_attn[2 * C :].astype(np.float64)
    bpe = (b_proj.astype(np.float64) + bv @ W_proj.astype(np.float64)).astype(
        np.float16
    )
    # mask bias per key position: col 0 -> query in image1, col 1 -> image2
    mbm = np.zeros((2, 512), dtype=np.float32)
    k = np.arange(T)
    img2 = (k >= T1 + 1).astype(np.float32)
    kzero = (k == 0).astype(np.float32)
    mbm[0, :T] = kzero + img2          # q in img1: mask 1 at k=0 and k in img2
    mbm[1, :T] = 1.0 - img2            # q in img2: mask 1 at k=0 and k in img1
    # device layout [p, kt, j]: mb_dev[p, kt, j] = mbm[j, kt*128 + p]
    mb_dev = np.ascontiguousarray(mbm.reshape(2, 4, 128).transpose(2, 1, 0))
    common = {
        "wqk": wqk, "wv": wv, "wp": wpk, "bqs": bqs, "bk": bk,
        "bpe": bpe, "mb": mb_dev,
    }
    # x -> [B, C, T] fp16 (pre-transposed so the device DMA is near-linear)
    xs = np.ascontiguousarray(
        np.asarray(x).astype(np.float16).transpose(0, 2, 1)
    )
    in_maps = []
    for cidx in range(NCORES):
        m = dict(common)
        m["x"] = np.ascontiguousarray(xs[cidx * BL : (cidx + 1) * BL])
        in_maps.append(m)
    return in_maps


def _run(x, W_attn, b_attn, W_proj, b_proj, trace=False):
    if "nc" not in _cache:
        _cache["nc"] = _build()
    nc = _cache["nc"]
    in_maps = _prep_inputs(x, W_attn, b_attn, W_proj, b_proj)
    res = run_bass_kernel_spmd(
        nc, in_maps, core_ids=list(range(NCORES)), trace=trace
    )
    out = np.concatenate([r["out"] for r in res.results], axis=0)
    return out.astype(np.float32), res


def kernel(x, W_attn, b_attn, W_proj, b_proj):
    out, _ = _run(x, W_attn, b_attn, W_proj, b_proj, trace=False)
    return out
